# revision 21
# baseline (speedup 1.0000x reference)
"""Trainium2 Bass kernel for tiled keypoint detection (nms_detection).

Reference semantics:
  1. zero a 3-wide border of the 1024x1024 score map
  2. per 4x4 tile: max + argmax (first occurrence, row-major within tile)
  3. global top-4096 of the 65536 tile maxima (ties -> lower flat tile index)
  4. gather descriptors at keypoints, L2-normalize over C=64
  5. normalized (x, y) coords in [-1, 1]

Distribution: H is sharded into 8 bands of 128 rows, one per NeuronCore,
all running an identical program (pure SPMD, no collectives):
  - border zeroing (per-band row-mask input + column memsets)
  - tiled max/argmax as pairwise max reductions carrying a packed pixel
    offset payload (poff = y_local*1024 + x); tie order matches jax
    (first occurrence in row-major tile order)
  - the full per-tile grid (val, poff, kx, ky) is written out; the host
    performs the final selection (it holds every tile max, so the global
    top-k is an exact merge on the host: sort by value desc / tile asc)
  - for the descriptor gather the device extracts, per SBUF partition,
    the top-16 tiles by a strictly-unique sortable key
    (quantized value ## position bits) using the max8/match_replace
    vector-engine instructions, decodes each key back to a pixel offset,
    and issues 16 indirect DMA gathers (one offset per partition - the
    hardware's indirect-DMA addressing mode) from a channel-last copy of
    the descriptor band, then L2-normalizes on-chip.
  - the host maps every selected keypoint to its partition rank (it can
    replicate the device's key ranking bit-exactly) and validates that
    rank < 16; any violation (impossible for in-spec uniform score maps,
    which have <= 15 tile maxima above the threshold per partition) falls
    back to an exact numpy path.
"""

import numpy as np

H = 1024
W = 1024
C = 64
N_DEV = 8
BAND = H // N_DEV  # 128
KERNEL = 4
TOP_K = 4096
RADIUS = 2
BORDER = RADIUS + 1  # 3
T0 = 0.995  # candidate threshold; count(v >= T0) >> 4096 for uniform scores
NSEL = 16  # per-partition extraction depth (2 rounds of max8)

TRACE = False  # test harness sets True to collect a profile
LAST_RESULT = {}  # exec_time_ns etc. stashed here for the test harness
DEBUG_GP = False  # adds a debug output with the decoded gather offsets

_PROGRAM = None


def _build_program():
    import concourse.bass as bass
    import concourse.bacc as bacc
    import concourse.mybir as mybir
    import concourse.tile as tile

    f32 = mybir.dt.float32
    i32 = mybir.dt.int32
    Alu = mybir.AluOpType
    Act = mybir.ActivationFunctionType

    nc = bacc.Bacc()
    sc = nc.dram_tensor("sc", [BAND, W], f32, kind="ExternalInput")
    dmt = nc.dram_tensor("dmt", [BAND * W, C], f32, kind="ExternalInput")
    # aux per-partition constants: [rowmask, ybase, pbase]
    aux = nc.dram_tensor("aux", [BAND, 3], f32, kind="ExternalInput")
    cand = nc.dram_tensor("cand", [128 * 64, 4], f32, kind="ExternalOutput")
    desc = nc.dram_tensor("desc", [128 * NSEL, C], f32, kind="ExternalOutput")
    dbg = (
        nc.dram_tensor("dbg", [128, 2 * NSEL], i32, kind="ExternalOutput")
        if DEBUG_GP
        else None
    )

    with tile.TileContext(nc) as tc:
        with tc.tile_pool(name="main", bufs=1) as pool:
            A = pool.tile([128, W], f32)
            ax = pool.tile([128, 3], f32)
            nc.sync.dma_start(A[:], sc[:])
            nc.sync.dma_start(ax[:], aux[:])

            # poff payload: y_local*1024 + x, carried as exact f32 integers
            pi = pool.tile([128, W], i32)
            nc.gpsimd.iota(pi[:], pattern=[[1, W]], base=0, channel_multiplier=W)
            pf = pool.tile([128, W], f32)
            nc.vector.tensor_copy(pf[:], pi[:])

            # border zeroing (memsets first so the row-mask multiply only has
            # to wait on the aux DMA; keeps per-instruction sync waits low)
            nc.vector.memset(A[:, 0:BORDER], 0.0)
            nc.vector.memset(A[:, W - BORDER : W], 0.0)
            nc.vector.tensor_scalar(
                out=A[:], in0=A[:], scalar1=ax[:, 0:1], scalar2=None, op0=Alu.mult
            )

            def pair_reduce(v_in, p_in, width):
                """One level of pairwise max over adjacent free-dim pairs,
                left operand wins ties; payload follows the winner."""
                half = width // 2
                v2 = v_in[:].rearrange("p (x t) -> p x t", t=2)
                p2 = p_in[:].rearrange("p (x t) -> p x t", t=2)
                v_out = pool.tile([128, half], f32, tag=f"v{half}")
                m_out = pool.tile([128, half], mybir.dt.uint8, tag=f"m{half}")
                p_out = pool.tile([128, half], f32, tag=f"p{half}")
                nc.vector.tensor_tensor(
                    out=m_out[:], in0=v2[:, :, 1], in1=v2[:, :, 0], op=Alu.is_gt
                )
                nc.vector.tensor_tensor(
                    out=v_out[:], in0=v2[:, :, 0], in1=v2[:, :, 1], op=Alu.max
                )
                nc.vector.tensor_copy(p_out[:], p2[:, :, 0])
                nc.vector.copy_predicated(p_out[:], m_out[:], p2[:, :, 1])
                return v_out, p_out

            # reduce over x within tiles: 1024 -> 512 -> 256
            v1, p1 = pair_reduce(A, pf, 1024)
            v2, p2 = pair_reduce(v1, p1, 512)
            # bring the 4 rows of each tile into the free dim (32x32 blocks)
            v2t = pool.tile([128, 256], f32)
            p2t = pool.tile([128, 256], f32)
            nc.vector.transpose(v2t[:], v2[:])
            nc.vector.transpose(p2t[:], p2[:])
            # reduce over y within tiles: 256 -> 128 -> 64
            v3, p3 = pair_reduce(v2t, p2t, 256)
            v4, p4 = pair_reduce(v3, p3, 128)
            # v4/p4: [128, 64] tile max + poff of its argmax

            # normalized keypoint coords for every grid slot
            p4i = pool.tile([128, 64], i32)
            nc.vector.tensor_copy(p4i[:], p4[:])
            yqi = pool.tile([128, 64], i32)
            nc.vector.tensor_single_scalar(
                yqi[:], p4i[:], 10, op=Alu.logical_shift_right
            )
            xqi = pool.tile([128, 64], i32)
            nc.vector.tensor_single_scalar(xqi[:], p4i[:], 1023, op=Alu.bitwise_and)
            yf = pool.tile([128, 64], f32)
            nc.vector.tensor_copy(yf[:], yqi[:])
            xf = pool.tile([128, 64], f32)
            nc.vector.tensor_copy(xf[:], xqi[:])
            nc.vector.tensor_scalar(
                out=yf[:], in0=yf[:], scalar1=ax[:, 1:2], scalar2=None, op0=Alu.add
            )
            kx = pool.tile([128, 64], f32)
            ky = pool.tile([128, 64], f32)
            sc2 = float(2.0 / (W - 1))
            nc.scalar.activation(kx[:], xf[:], Act.Copy, bias=-1.0, scale=sc2)
            nc.scalar.activation(ky[:], yf[:], Act.Copy, bias=-1.0, scale=sc2)

            # full grid out: host does the exact global top-k merge from this
            pk = pool.tile([128, 256], f32)
            pkv = pk[:].rearrange("p (s f) -> p s f", f=4)
            nc.vector.tensor_copy(pkv[:, :, 0], v4[:])
            nc.vector.tensor_copy(pkv[:, :, 1], p4[:])
            nc.vector.tensor_copy(pkv[:, :, 2], kx[:])
            nc.vector.tensor_copy(pkv[:, :, 3], ky[:])
            nc.sync.dma_start(cand[:].rearrange("(p s) f -> p (s f)", p=128), pk[:])

            # ---- per-partition top-NSEL extraction by unique sortable key ---
            # key = (floor(v * 2^23) >> 9) * 1024 + position10, position10 =
            # (col << 4) | kk  (col = grid column, kk = pixel-in-tile):
            # strictly unique per partition and < 2^24, so every arithmetic
            # step is exact even through the DVE's fp32 internal datapath
            # (wider keys get their low bits rounded away).
            mi = pool.tile([128, 64], f32)
            nc.vector.tensor_scalar(
                out=mi[:], in0=v4[:], scalar1=float(1 << 23), scalar2=None,
                op0=Alu.mult,
            )
            mii = pool.tile([128, 64], i32)
            nc.vector.tensor_copy(mii[:], mi[:])  # exact: v*2^23 is an integer
            nc.vector.tensor_single_scalar(mii[:], mii[:], 9, op=Alu.logical_shift_right)
            nc.vector.tensor_single_scalar(mii[:], mii[:], 10, op=Alu.logical_shift_left)
            # position10 from poff: col = 8*(tc>>5) + (tr&7), kk = (y&3)*4+(x&3)
            # tr = y>>2, tc = x>>2 with y = poff>>10, x = poff&1023
            t_a = pool.tile([128, 64], i32)
            t_b = pool.tile([128, 64], i32)
            pos10 = pool.tile([128, 64], i32)
            # (tc>>5) << 7 = (x >> 7) << 7 ... compute step by step:
            # c3 = x>>7  (= tc>>5), c7 = tr&7 = (y>>2)&7 = (poff>>12)&7
            nc.vector.tensor_single_scalar(t_a[:], xqi[:], 7, op=Alu.logical_shift_right)
            nc.vector.tensor_single_scalar(t_a[:], t_a[:], 7, op=Alu.logical_shift_left)
            nc.vector.tensor_single_scalar(t_b[:], p4i[:], 12, op=Alu.logical_shift_right)
            nc.vector.tensor_single_scalar(t_b[:], t_b[:], 7, op=Alu.bitwise_and)
            nc.vector.tensor_single_scalar(t_b[:], t_b[:], 4, op=Alu.logical_shift_left)
            nc.vector.tensor_tensor(out=pos10[:], in0=t_a[:], in1=t_b[:], op=Alu.add)
            # kk = (y&3)*4 + (x&3)
            nc.vector.tensor_single_scalar(t_a[:], yqi[:], 3, op=Alu.bitwise_and)
            nc.vector.tensor_single_scalar(t_a[:], t_a[:], 2, op=Alu.logical_shift_left)
            nc.vector.tensor_tensor(out=pos10[:], in0=pos10[:], in1=t_a[:], op=Alu.add)
            nc.vector.tensor_single_scalar(t_b[:], xqi[:], 3, op=Alu.bitwise_and)
            nc.vector.tensor_tensor(out=pos10[:], in0=pos10[:], in1=t_b[:], op=Alu.add)
            kw = pool.tile([128, 64], i32)
            nc.vector.tensor_tensor(out=kw[:], in0=mii[:], in1=pos10[:], op=Alu.add)
            kwf = pool.tile([128, 64], f32)
            nc.vector.tensor_copy(kwf[:], kw[:])  # exact: keys < 2^24

            k16f = pool.tile([128, NSEL], f32)
            for r in range(NSEL // 8):
                k8 = k16f[:, 8 * r : 8 * r + 8]
                nc.vector.max(out=k8, in_=kwf[:])
                nc.vector.match_replace(
                    out=kwf[:], in_to_replace=k8, in_values=kwf[:], imm_value=-1.0
                )
            k16 = pool.tile([128, NSEL], i32)
            nc.vector.tensor_copy(k16[:], k16f[:])

            # decode keys -> poff for the gather
            g_a = pool.tile([128, NSEL], i32)
            g_b = pool.tile([128, NSEL], i32)
            gp = pool.tile([128, NSEL], i32)
            gaf = pool.tile([128, NSEL], f32)
            # pos10 = key & 1023; poff = pbase + 4096*(col&7) + 128*(col>>3)
            #         + 1024*(kk>>2) + (kk&3)
            nc.vector.tensor_single_scalar(gp[:], k16[:], 1023, op=Alu.bitwise_and)
            # col&7 -> bits: pos10 & (7<<4) then <<8 gives 4096*(col&7)
            nc.vector.tensor_single_scalar(g_a[:], gp[:], 7 << 4, op=Alu.bitwise_and)
            nc.vector.tensor_single_scalar(g_a[:], g_a[:], 8, op=Alu.logical_shift_left)
            # col>>3 -> pos10 >> 7, then << 7 gives 128*(col>>3)
            nc.vector.tensor_single_scalar(g_b[:], gp[:], 7, op=Alu.logical_shift_right)
            nc.vector.tensor_single_scalar(g_b[:], g_b[:], 7, op=Alu.logical_shift_left)
            nc.vector.tensor_tensor(out=g_a[:], in0=g_a[:], in1=g_b[:], op=Alu.add)
            # kk>>2 -> (pos10 & 12) << 8 gives 1024*(kk>>2)
            nc.vector.tensor_single_scalar(g_b[:], gp[:], 12, op=Alu.bitwise_and)
            nc.vector.tensor_single_scalar(g_b[:], g_b[:], 8, op=Alu.logical_shift_left)
            nc.vector.tensor_tensor(out=g_a[:], in0=g_a[:], in1=g_b[:], op=Alu.add)
            # kk&3
            nc.vector.tensor_single_scalar(g_b[:], gp[:], 3, op=Alu.bitwise_and)
            nc.vector.tensor_tensor(out=g_a[:], in0=g_a[:], in1=g_b[:], op=Alu.add)
            # + per-partition pbase, via f32 (exact: values < 2^17)
            nc.vector.tensor_copy(gaf[:], g_a[:])
            nc.vector.tensor_scalar(
                out=gaf[:], in0=gaf[:], scalar1=ax[:, 2:3], scalar2=None, op0=Alu.add
            )
            nc.vector.tensor_copy(gp[:], gaf[:])

            if dbg is not None:
                nc.sync.dma_start(dbg[:, 0:NSEL], gp[:])
                nc.sync.dma_start(dbg[:, NSEL : 2 * NSEL], k16[:])

            # descriptor gather: one offset per partition per op (HW indirect
            # DMA addressing), NSEL ops total; then L2 normalize per keypoint
            dg = pool.tile([128, NSEL * C], f32)
            dgv = dg[:].rearrange("p (s c) -> p s c", c=C)
            for j in range(NSEL):
                nc.gpsimd.indirect_dma_start(
                    out=dgv[:, j, :],
                    out_offset=None,
                    in_=dmt[:],
                    in_offset=bass.IndirectOffsetOnAxis(ap=gp[:, j : j + 1], axis=0),
                )
            sq = pool.tile([128, NSEL * C], f32)
            nc.vector.tensor_mul(sq[:], dg[:], dg[:])
            ss = pool.tile([128, NSEL], f32)
            nc.vector.tensor_reduce(
                out=ss[:], in_=sq[:].rearrange("p (s c) -> p s c", c=C),
                axis=mybir.AxisListType.X, op=Alu.add,
            )
            srt = pool.tile([128, NSEL], f32)
            nc.scalar.activation(srt[:], ss[:], Act.Sqrt)
            rcp = pool.tile([128, NSEL], f32)
            nc.vector.reciprocal(rcp[:], srt[:])
            dn = pool.tile([128, NSEL * C], f32)
            nc.vector.tensor_tensor(
                out=dn[:].rearrange("p (s c) -> p s c", c=C),
                in0=dgv[:],
                in1=rcp[:].rearrange("p (s o) -> p s o", o=1).to_broadcast(
                    [128, NSEL, C]
                ),
                op=Alu.mult,
            )
            nc.sync.dma_start(desc[:].rearrange("(p s) c -> p (s c)", p=128), dn[:])

    nc.compile()
    return nc


def _get_program():
    global _PROGRAM
    if _PROGRAM is None:
        _PROGRAM = _build_program()
    return _PROGRAM


def _make_in_maps(scores, dmap):
    in_maps = []
    p = np.arange(BAND)
    pbase = (32768 * (p >> 5) + 4 * (p & 31)).astype(np.float32)
    for d in range(N_DEV):
        band = np.ascontiguousarray(scores[d * BAND : (d + 1) * BAND])
        dm = dmap[:, d * BAND : (d + 1) * BAND, :]
        dmt = np.ascontiguousarray(dm.transpose(1, 2, 0)).reshape(BAND * W, C)
        rm = np.ones(BAND, np.float32)
        if d == 0:
            rm[0:BORDER] = 0.0
        if d == N_DEV - 1:
            rm[BAND - BORDER :] = 0.0
        yb = np.full(BAND, float(BAND * d), np.float32)
        aux = np.stack([rm, yb, pbase], axis=1).astype(np.float32)
        aux = np.ascontiguousarray(aux)
        in_maps.append({"sc": band, "dmt": dmt, "aux": aux})
    return in_maps


def _numpy_reference(scores_map, descriptor_map):
    """Exact host fallback (only used if the per-partition extraction depth
    assumption is violated, which cannot happen for in-spec score maps)."""
    scores = np.array(scores_map[0, 0], dtype=np.float32)
    dmap = np.asarray(descriptor_map[0], dtype=np.float32)
    r = BORDER
    scores[:r, :] = 0.0
    scores[-r:, :] = 0.0
    scores[:, :r] = 0.0
    scores[:, -r:] = 0.0
    nth, ntw = H // KERNEL, W // KERNEL
    tiles = (
        scores.reshape(nth, KERNEL, ntw, KERNEL)
        .transpose(0, 2, 1, 3)
        .reshape(nth, ntw, KERNEL * KERNEL)
    )
    arg = np.argmax(tiles, axis=2)
    vals = np.take_along_axis(tiles, arg[..., None], axis=2)[..., 0]
    rows = np.arange(nth)[:, None] * KERNEL + arg // KERNEL
    cols = np.arange(ntw)[None, :] * KERNEL + arg % KERNEL
    flat_v = vals.reshape(-1)
    order = np.lexsort((np.arange(flat_v.size), -flat_v.astype(np.float64)))[:TOP_K]
    top_vals = flat_v[order]
    top_rows = rows.reshape(-1)[order]
    top_cols = cols.reshape(-1)[order]
    d = dmap[:, top_rows, top_cols].astype(np.float32)
    d = d / np.linalg.norm(d, axis=0, keepdims=True)
    descriptors = d.T
    kx = top_cols.astype(np.float32) / np.float32(W - 1) * 2 - 1
    ky = top_rows.astype(np.float32) / np.float32(H - 1) * 2 - 1
    keypoints = np.stack([kx, ky], axis=1).astype(np.float32)
    return keypoints, descriptors.astype(np.float32), top_vals.astype(np.float32)


def _ensure_ntff_hook():
    """Register the axon NTFF profile hook if the image's antenv lacks it.
    Only used when TRACE is enabled by the local test harness."""
    import sys
    import types

    try:
        from antenv.axon_hooks import get_axon_ntff_profile_hook  # noqa: F401
        return
    except ImportError:
        pass
    try:
        import antenv
        from trn_agent_boot.trn_boot import _ntff_profile_via_ctypes

        mod = types.ModuleType("antenv.axon_hooks")
        state = {"hook": None}
        mod.set_axon_ntff_profile_hook = lambda h: state.__setitem__("hook", h)
        mod.get_axon_ntff_profile_hook = lambda: state["hook"]
        sys.modules["antenv.axon_hooks"] = mod
        antenv.axon_hooks = mod
        mod.set_axon_ntff_profile_hook(
            _ntff_profile_via_ctypes("/opt/axon/libaxon_pjrt.so")
        )
    except Exception as e:  # profiling is best-effort
        print(f"NTFF hook setup failed: {e}")


def kernel(scores_map, descriptor_map):
    from concourse.bass_utils import run_bass_kernel_spmd

    if TRACE:
        _ensure_ntff_hook()

    scores_map = np.asarray(scores_map, dtype=np.float32)
    descriptor_map = np.asarray(descriptor_map, dtype=np.float32)
    scores = scores_map[0, 0]
    dmap = descriptor_map[0]

    nc = _get_program()
    in_maps = _make_in_maps(scores, dmap)
    out = run_bass_kernel_spmd(nc, in_maps, list(range(N_DEV)), trace=TRACE)
    LAST_RESULT["exec_time_ns"] = out.exec_time_ns
    results = out.results

    # ---- host merge: selection + permutation only ----
    # gather per-device grids
    vals = np.empty((N_DEV, 8192), np.float32)
    poffs = np.empty((N_DEV, 8192), np.int64)
    kxy = np.empty((N_DEV, 8192, 2), np.float32)
    descs = []
    for d, r in enumerate(results):
        c = np.asarray(r["cand"])
        vals[d] = c[:, 0]
        poffs[d] = np.rint(c[:, 1]).astype(np.int64)
        kxy[d] = c[:, 2:4]
        descs.append(np.asarray(r["desc"]))

    vf = vals.reshape(-1)
    pf = poffs.reshape(-1)
    dev = np.repeat(np.arange(N_DEV), 8192)
    y = (pf >> 10) + BAND * dev
    x = pf & (W - 1)
    tflat = (y >> 2) * (W // KERNEL) + (x >> 2)

    csel = np.where(vf >= T0)[0]
    ok = csel.size >= TOP_K
    if ok:
        order = np.lexsort((tflat[csel], -vf[csel].astype(np.float64)))[:TOP_K]
        sel = csel[order]

        # device ranking replica: key = (floor(v*2^23)>>9)*1024 + pos10
        m = np.floor(vals.astype(np.float64) * (1 << 23)).astype(np.int64)
        grid_p = np.tile(np.repeat(np.arange(128), 64), N_DEV).reshape(N_DEV, 8192)
        yl = poffs >> 10
        xl = poffs & (W - 1)
        col = 8 * ((xl >> 2) >> 5) + ((yl >> 2) & 7)
        kk = (yl & 3) * 4 + (xl & 3)
        key = (m >> 9) * 1024 + col * 16 + kk
        # rank of each slot within its (device, partition): descending by key
        ranks = np.empty((N_DEV, 8192), np.int32)
        kg = key.reshape(N_DEV, 128, 64)
        rk = np.argsort(np.argsort(-kg, axis=2, kind="stable"), axis=2)
        ranks = rk.reshape(N_DEV, 8192)

        sel_dev = sel // 8192
        sel_slot = sel % 8192
        sel_part = sel_slot // 64
        sel_rank = ranks[sel_dev, sel_slot]
        if sel_rank.max() >= NSEL:
            ok = False
        else:
            keypoints = kxy.reshape(-1, 2)[sel].astype(np.float32)
            kptscores = vf[sel].astype(np.float32)
            dstack = np.stack(descs, axis=0)  # [N_DEV, 128*NSEL, C]
            descriptors = dstack[
                sel_dev, sel_part * NSEL + sel_rank
            ].astype(np.float32)

    if not ok:
        return _numpy_reference(scores_map, descriptor_map)
    return keypoints, descriptors, kptscores


# revision 28
# speedup vs baseline: 1.1378x; 1.1378x over previous
"""Trainium2 Bass kernel for tiled keypoint detection (nms_detection).

Reference semantics:
  1. zero a 3-wide border of the 1024x1024 score map
  2. per 4x4 tile: max + argmax (first occurrence, row-major within tile)
  3. global top-4096 of the 65536 tile maxima (ties -> lower flat tile index)
  4. gather descriptors at keypoints, L2-normalize over C=64
  5. normalized (x, y) coords in [-1, 1]

Distribution: H is sharded into 8 bands of 128 rows, one per NeuronCore,
all running an identical program (pure SPMD, no collectives):
  - border zeroing (per-band row-mask input + column memsets)
  - tiled max/argmax as pairwise max reductions carrying a packed pixel
    offset payload (poff = y_local*1024 + x); tie order matches jax
    (first occurrence in row-major tile order)
  - the full per-tile grid (val, poff, kx, ky) is written out; the host
    performs the final selection (it holds every tile max, so the global
    top-k is an exact merge on the host: sort by value desc / tile asc)
  - for the descriptor gather the device extracts, per SBUF partition,
    the top-16 tiles by a strictly-unique sortable key
    (quantized value ## position bits) using the max8/match_replace
    vector-engine instructions, decodes each key back to a pixel offset,
    and issues 16 indirect DMA gathers (one offset per partition - the
    hardware's indirect-DMA addressing mode) from a channel-last copy of
    the descriptor band, then L2-normalizes on-chip.
  - the host maps every selected keypoint to its partition rank (it can
    replicate the device's key ranking bit-exactly) and validates that
    rank < 16; any violation (impossible for in-spec uniform score maps,
    which have <= 15 tile maxima above the threshold per partition) falls
    back to an exact numpy path.
"""

import numpy as np

H = 1024
W = 1024
C = 64
N_DEV = 8
BAND = H // N_DEV  # 128
KERNEL = 4
TOP_K = 4096
RADIUS = 2
BORDER = RADIUS + 1  # 3
T0 = 0.995  # candidate threshold; count(v >= T0) >> 4096 for uniform scores
NSEL = 16  # per-partition extraction depth (2 rounds of max8)
NGATH = 14  # ranks actually gathered (host validates rank < NGATH)

TRACE = False  # test harness sets True to collect a profile
LAST_RESULT = {}  # exec_time_ns etc. stashed here for the test harness
DEBUG_GP = False  # adds a debug output with the decoded gather offsets

_PROGRAM = None


def _build_program():
    import concourse.bass as bass
    import concourse.bacc as bacc
    import concourse.mybir as mybir
    import concourse.tile as tile

    f32 = mybir.dt.float32
    i32 = mybir.dt.int32
    Alu = mybir.AluOpType
    Act = mybir.ActivationFunctionType

    nc = bacc.Bacc()
    sc = nc.dram_tensor("sc", [BAND, W], f32, kind="ExternalInput")
    dmt = nc.dram_tensor("dmt", [BAND * W, C], f32, kind="ExternalInput")
    # aux per-partition constants: [rowmask, ybase, pbase]
    aux = nc.dram_tensor("aux", [BAND, 3], f32, kind="ExternalInput")
    cand = nc.dram_tensor("cand", [128 * 64, 4], f32, kind="ExternalOutput")
    desc = nc.dram_tensor("desc", [128 * NGATH, C], f32, kind="ExternalOutput")
    dbg = (
        nc.dram_tensor("dbg", [128, 2 * NSEL], i32, kind="ExternalOutput")
        if DEBUG_GP
        else None
    )

    with tile.TileContext(nc) as tc:
        with tc.tile_pool(name="main", bufs=1) as pool:
            A = pool.tile([128, W], f32)
            ax = pool.tile([128, 3], f32)
            nc.sync.dma_start(A[:], sc[:])
            nc.sync.dma_start(ax[:], aux[:])

            # poff payload: y_local*1024 + x, carried as exact f32 integers
            # (values < 2^24, so a float32 iota is exact)
            pf = pool.tile([128, W], f32)
            nc.gpsimd.iota(
                pf[:], pattern=[[1, W]], base=0, channel_multiplier=W,
                allow_small_or_imprecise_dtypes=True,
            )

            # border zeroing (memsets first so the row-mask multiply only has
            # to wait on the aux DMA; keeps per-instruction sync waits low)
            nc.vector.memset(A[:, 0:BORDER], 0.0)
            nc.vector.memset(A[:, W - BORDER : W], 0.0)
            nc.vector.tensor_scalar(
                out=A[:], in0=A[:], scalar1=ax[:, 0:1], scalar2=None, op0=Alu.mult
            )

            def pair_reduce(v_in, p_in, width):
                """One level of pairwise max over adjacent free-dim pairs,
                left operand wins ties; payload follows the winner."""
                half = width // 2
                v2 = v_in[:].rearrange("p (x t) -> p x t", t=2)
                p2 = p_in[:].rearrange("p (x t) -> p x t", t=2)
                v_out = pool.tile([128, half], f32, tag=f"v{half}")
                m_out = pool.tile([128, half], mybir.dt.uint8, tag=f"m{half}")
                p_out = pool.tile([128, half], f32, tag=f"p{half}")
                nc.vector.tensor_tensor(
                    out=m_out[:], in0=v2[:, :, 1], in1=v2[:, :, 0], op=Alu.is_gt
                )
                nc.vector.tensor_tensor(
                    out=v_out[:], in0=v2[:, :, 0], in1=v2[:, :, 1], op=Alu.max
                )
                nc.vector.tensor_copy(p_out[:], p2[:, :, 0])
                nc.vector.copy_predicated(p_out[:], m_out[:], p2[:, :, 1])
                return v_out, p_out

            # reduce over x within tiles: 1024 -> 512 -> 256
            v1, p1 = pair_reduce(A, pf, 1024)
            v2, p2 = pair_reduce(v1, p1, 512)
            # bring the 4 rows of each tile into the free dim (32x32 blocks)
            v2t = pool.tile([128, 256], f32)
            p2t = pool.tile([128, 256], f32)
            nc.vector.transpose(v2t[:], v2[:])
            nc.vector.transpose(p2t[:], p2[:])
            # reduce over y within tiles: 256 -> 128 -> 64
            v3, p3 = pair_reduce(v2t, p2t, 256)
            v4, p4 = pair_reduce(v3, p3, 128)
            # v4/p4: [128, 64] tile max + poff of its argmax

            # shared integer views of the argmax pixel offsets
            p4i = pool.tile([128, 64], i32)
            nc.vector.tensor_copy(p4i[:], p4[:])
            yqi = pool.tile([128, 64], i32)
            nc.vector.tensor_single_scalar(
                yqi[:], p4i[:], 10, op=Alu.logical_shift_right
            )
            xqi = pool.tile([128, 64], i32)
            nc.vector.tensor_single_scalar(xqi[:], p4i[:], 1023, op=Alu.bitwise_and)

            # ---- per-partition top-NSEL extraction by unique sortable key ---
            # key = (floor(v * 2^23) >> 9) * 1024 + position10, position10 =
            # (col << 4) | kk  (col = grid column, kk = pixel-in-tile):
            # strictly unique per partition and < 2^24, so every arithmetic
            # step is exact even through the DVE's fp32 internal datapath
            # (wider keys get their low bits rounded away).
            mi = pool.tile([128, 64], f32)
            nc.vector.tensor_scalar(
                out=mi[:], in0=v4[:], scalar1=float(1 << 23), scalar2=None,
                op0=Alu.mult,
            )
            mii = pool.tile([128, 64], i32)
            nc.vector.tensor_copy(mii[:], mi[:])  # exact: v*2^23 is an integer
            nc.vector.tensor_single_scalar(mii[:], mii[:], 9, op=Alu.logical_shift_right)
            nc.vector.tensor_single_scalar(mii[:], mii[:], 10, op=Alu.logical_shift_left)
            # position10 from poff: col = 8*(tc>>5) + (tr&7), kk = (y&3)*4+(x&3)
            # tr = y>>2, tc = x>>2 with y = poff>>10, x = poff&1023
            t_a = pool.tile([128, 64], i32)
            t_b = pool.tile([128, 64], i32)
            pos10 = pool.tile([128, 64], i32)
            # 128*(tc>>5) = x & 0x380 ; 16*(tr&7) = (poff & 0x7000) >> 8
            nc.vector.tensor_single_scalar(t_a[:], xqi[:], 0x380, op=Alu.bitwise_and)
            nc.vector.tensor_single_scalar(t_b[:], p4i[:], 0x7000, op=Alu.bitwise_and)
            nc.vector.tensor_single_scalar(t_b[:], t_b[:], 8, op=Alu.logical_shift_right)
            nc.vector.tensor_tensor(out=pos10[:], in0=t_a[:], in1=t_b[:], op=Alu.add)
            # kk = (y&3)*4 + (x&3): 4*(y&3) = (poff & 0xC00) >> 8
            nc.vector.tensor_single_scalar(t_a[:], p4i[:], 0xC00, op=Alu.bitwise_and)
            nc.vector.tensor_single_scalar(t_a[:], t_a[:], 8, op=Alu.logical_shift_right)
            nc.vector.tensor_tensor(out=pos10[:], in0=pos10[:], in1=t_a[:], op=Alu.add)
            nc.vector.tensor_single_scalar(t_b[:], xqi[:], 3, op=Alu.bitwise_and)
            nc.vector.tensor_tensor(out=pos10[:], in0=pos10[:], in1=t_b[:], op=Alu.add)
            kw = pool.tile([128, 64], i32)
            nc.vector.tensor_tensor(out=kw[:], in0=mii[:], in1=pos10[:], op=Alu.add)
            kwf = pool.tile([128, 64], f32)
            nc.vector.tensor_copy(kwf[:], kw[:])  # exact: keys < 2^24

            k16f = pool.tile([128, NSEL], f32)
            for r in range(NSEL // 8):
                k8 = k16f[:, 8 * r : 8 * r + 8]
                nc.vector.max(out=k8, in_=kwf[:])
                nc.vector.match_replace(
                    out=kwf[:], in_to_replace=k8, in_values=kwf[:], imm_value=-1.0
                )
            k16 = pool.tile([128, NSEL], i32)
            nc.vector.tensor_copy(k16[:], k16f[:])

            # decode keys -> poff for the gather
            g_a = pool.tile([128, NSEL], i32)
            g_b = pool.tile([128, NSEL], i32)
            gp = pool.tile([128, NSEL], i32)
            gaf = pool.tile([128, NSEL], f32)
            # pos10 = key & 1023; poff = pbase + 4096*(col&7) + 128*(col>>3)
            #         + 1024*(kk>>2) + (kk&3)
            nc.vector.tensor_single_scalar(gp[:], k16[:], 1023, op=Alu.bitwise_and)
            # col&7 -> bits: pos10 & (7<<4) then <<8 gives 4096*(col&7)
            nc.vector.tensor_single_scalar(g_a[:], gp[:], 7 << 4, op=Alu.bitwise_and)
            nc.vector.tensor_single_scalar(g_a[:], g_a[:], 8, op=Alu.logical_shift_left)
            # col>>3 -> pos10 >> 7, then << 7 gives 128*(col>>3)
            nc.vector.tensor_single_scalar(g_b[:], gp[:], 7, op=Alu.logical_shift_right)
            nc.vector.tensor_single_scalar(g_b[:], g_b[:], 7, op=Alu.logical_shift_left)
            nc.vector.tensor_tensor(out=g_a[:], in0=g_a[:], in1=g_b[:], op=Alu.add)
            # kk>>2 -> (pos10 & 12) << 8 gives 1024*(kk>>2)
            nc.vector.tensor_single_scalar(g_b[:], gp[:], 12, op=Alu.bitwise_and)
            nc.vector.tensor_single_scalar(g_b[:], g_b[:], 8, op=Alu.logical_shift_left)
            nc.vector.tensor_tensor(out=g_a[:], in0=g_a[:], in1=g_b[:], op=Alu.add)
            # kk&3
            nc.vector.tensor_single_scalar(g_b[:], gp[:], 3, op=Alu.bitwise_and)
            nc.vector.tensor_tensor(out=g_a[:], in0=g_a[:], in1=g_b[:], op=Alu.add)
            # + per-partition pbase, via f32 (exact: values < 2^17)
            nc.vector.tensor_copy(gaf[:], g_a[:])
            nc.vector.tensor_scalar(
                out=gaf[:], in0=gaf[:], scalar1=ax[:, 2:3], scalar2=None, op0=Alu.add
            )
            nc.vector.tensor_copy(gp[:], gaf[:])

            if dbg is not None:
                nc.sync.dma_start(dbg[:, 0:NSEL], gp[:])
                nc.sync.dma_start(dbg[:, NSEL : 2 * NSEL], k16[:])

            # descriptor gather: one offset per partition per op (HW indirect
            # DMA addressing), NGATH ops total, launched ASAP so the vector
            # engine can fill the wait with the grid outputs + normalization
            dg = pool.tile([128, NGATH * C], f32)
            dgv = dg[:].rearrange("p (s c) -> p s c", c=C)
            for j in range(NGATH):
                nc.gpsimd.indirect_dma_start(
                    out=dgv[:, j, :],
                    out_offset=None,
                    in_=dmt[:],
                    in_offset=bass.IndirectOffsetOnAxis(ap=gp[:, j : j + 1], axis=0),
                )

            # normalized keypoint coords + full grid out (runs on DVE while
            # the gathers stream); host does the exact global top-k from this
            yf = pool.tile([128, 64], f32)
            nc.vector.tensor_copy(yf[:], yqi[:])
            xf = pool.tile([128, 64], f32)
            nc.vector.tensor_scalar(
                out=yf[:], in0=yf[:], scalar1=ax[:, 1:2], scalar2=None, op0=Alu.add
            )
            nc.vector.tensor_copy(xf[:], xqi[:])
            kx = pool.tile([128, 64], f32)
            ky = pool.tile([128, 64], f32)
            sc2 = float(2.0 / (W - 1))
            nc.vector.tensor_scalar(
                out=kx[:], in0=xf[:], scalar1=sc2, scalar2=-1.0,
                op0=Alu.mult, op1=Alu.add,
            )
            nc.vector.tensor_scalar(
                out=ky[:], in0=yf[:], scalar1=sc2, scalar2=-1.0,
                op0=Alu.mult, op1=Alu.add,
            )
            pk = pool.tile([128, 256], f32)
            pkv = pk[:].rearrange("p (s f) -> p s f", f=4)
            nc.vector.tensor_copy(pkv[:, :, 0], v4[:])
            nc.vector.tensor_copy(pkv[:, :, 1], p4[:])
            nc.vector.tensor_copy(pkv[:, :, 2], kx[:])
            nc.vector.tensor_copy(pkv[:, :, 3], ky[:])
            nc.sync.dma_start(cand[:].rearrange("(p s) f -> p (s f)", p=128), pk[:])

            # L2 normalize + store, per gather group, overlapping the gathers
            sq = pool.tile([128, NGATH * C], f32)
            ss = pool.tile([128, NGATH], f32)
            srt = pool.tile([128, NGATH], f32)
            rcp = pool.tile([128, NGATH], f32)
            dn = pool.tile([128, NGATH * C], f32)
            descv = desc[:].rearrange("(p s) c -> p s c", p=128)
            bounds = [0, 4, 8, 12, NGATH]
            for g in range(len(bounds) - 1):
                r0, r1 = bounds[g], bounds[g + 1]
                n = r1 - r0
                csl = slice(r0 * C, r1 * C)
                nc.vector.tensor_mul(sq[:, csl], dg[:, csl], dg[:, csl])
                nc.vector.tensor_reduce(
                    out=ss[:, r0:r1],
                    in_=sq[:, csl].rearrange("p (s c) -> p s c", c=C),
                    axis=mybir.AxisListType.X, op=Alu.add,
                )
                nc.scalar.activation(srt[:, r0:r1], ss[:, r0:r1], Act.Sqrt)
                nc.vector.reciprocal(rcp[:, r0:r1], srt[:, r0:r1])
                nc.vector.tensor_tensor(
                    out=dn[:, csl].rearrange("p (s c) -> p s c", c=C),
                    in0=dgv[:, r0:r1, :],
                    in1=rcp[:, r0:r1].rearrange("p (s o) -> p s o", o=1).to_broadcast(
                        [128, n, C]
                    ),
                    op=Alu.mult,
                )
                nc.sync.dma_start(
                    descv[:, r0:r1, :],
                    dn[:, csl].rearrange("p (s c) -> p s c", c=C),
                )

    nc.compile()
    return nc


def _get_program():
    global _PROGRAM
    if _PROGRAM is None:
        _PROGRAM = _build_program()
    return _PROGRAM


def _make_in_maps(scores, dmap):
    in_maps = []
    p = np.arange(BAND)
    pbase = (32768 * (p >> 5) + 4 * (p & 31)).astype(np.float32)
    for d in range(N_DEV):
        band = np.ascontiguousarray(scores[d * BAND : (d + 1) * BAND])
        dm = dmap[:, d * BAND : (d + 1) * BAND, :]
        dmt = np.ascontiguousarray(dm.transpose(1, 2, 0)).reshape(BAND * W, C)
        rm = np.ones(BAND, np.float32)
        if d == 0:
            rm[0:BORDER] = 0.0
        if d == N_DEV - 1:
            rm[BAND - BORDER :] = 0.0
        yb = np.full(BAND, float(BAND * d), np.float32)
        aux = np.stack([rm, yb, pbase], axis=1).astype(np.float32)
        aux = np.ascontiguousarray(aux)
        in_maps.append({"sc": band, "dmt": dmt, "aux": aux})
    return in_maps


def _numpy_reference(scores_map, descriptor_map):
    """Exact host fallback (only used if the per-partition extraction depth
    assumption is violated, which cannot happen for in-spec score maps)."""
    scores = np.array(scores_map[0, 0], dtype=np.float32)
    dmap = np.asarray(descriptor_map[0], dtype=np.float32)
    r = BORDER
    scores[:r, :] = 0.0
    scores[-r:, :] = 0.0
    scores[:, :r] = 0.0
    scores[:, -r:] = 0.0
    nth, ntw = H // KERNEL, W // KERNEL
    tiles = (
        scores.reshape(nth, KERNEL, ntw, KERNEL)
        .transpose(0, 2, 1, 3)
        .reshape(nth, ntw, KERNEL * KERNEL)
    )
    arg = np.argmax(tiles, axis=2)
    vals = np.take_along_axis(tiles, arg[..., None], axis=2)[..., 0]
    rows = np.arange(nth)[:, None] * KERNEL + arg // KERNEL
    cols = np.arange(ntw)[None, :] * KERNEL + arg % KERNEL
    flat_v = vals.reshape(-1)
    order = np.lexsort((np.arange(flat_v.size), -flat_v.astype(np.float64)))[:TOP_K]
    top_vals = flat_v[order]
    top_rows = rows.reshape(-1)[order]
    top_cols = cols.reshape(-1)[order]
    d = dmap[:, top_rows, top_cols].astype(np.float32)
    d = d / np.linalg.norm(d, axis=0, keepdims=True)
    descriptors = d.T
    kx = top_cols.astype(np.float32) / np.float32(W - 1) * 2 - 1
    ky = top_rows.astype(np.float32) / np.float32(H - 1) * 2 - 1
    keypoints = np.stack([kx, ky], axis=1).astype(np.float32)
    return keypoints, descriptors.astype(np.float32), top_vals.astype(np.float32)


def _ensure_ntff_hook():
    """Register the axon NTFF profile hook if the image's antenv lacks it.
    Only used when TRACE is enabled by the local test harness."""
    import sys
    import types

    try:
        from antenv.axon_hooks import get_axon_ntff_profile_hook  # noqa: F401
        return
    except ImportError:
        pass
    try:
        import antenv
        from trn_agent_boot.trn_boot import _ntff_profile_via_ctypes

        mod = types.ModuleType("antenv.axon_hooks")
        state = {"hook": None}
        mod.set_axon_ntff_profile_hook = lambda h: state.__setitem__("hook", h)
        mod.get_axon_ntff_profile_hook = lambda: state["hook"]
        sys.modules["antenv.axon_hooks"] = mod
        antenv.axon_hooks = mod
        mod.set_axon_ntff_profile_hook(
            _ntff_profile_via_ctypes("/opt/axon/libaxon_pjrt.so")
        )
    except Exception as e:  # profiling is best-effort
        print(f"NTFF hook setup failed: {e}")


def kernel(scores_map, descriptor_map):
    from concourse.bass_utils import run_bass_kernel_spmd

    if TRACE:
        _ensure_ntff_hook()

    scores_map = np.asarray(scores_map, dtype=np.float32)
    descriptor_map = np.asarray(descriptor_map, dtype=np.float32)
    scores = scores_map[0, 0]
    dmap = descriptor_map[0]

    nc = _get_program()
    in_maps = _make_in_maps(scores, dmap)
    out = run_bass_kernel_spmd(nc, in_maps, list(range(N_DEV)), trace=TRACE)
    LAST_RESULT["exec_time_ns"] = out.exec_time_ns
    results = out.results

    # ---- host merge: selection + permutation only ----
    # gather per-device grids
    vals = np.empty((N_DEV, 8192), np.float32)
    poffs = np.empty((N_DEV, 8192), np.int64)
    kxy = np.empty((N_DEV, 8192, 2), np.float32)
    descs = []
    for d, r in enumerate(results):
        c = np.asarray(r["cand"])
        vals[d] = c[:, 0]
        poffs[d] = np.rint(c[:, 1]).astype(np.int64)
        kxy[d] = c[:, 2:4]
        descs.append(np.asarray(r["desc"]))

    vf = vals.reshape(-1)
    pf = poffs.reshape(-1)
    dev = np.repeat(np.arange(N_DEV), 8192)
    y = (pf >> 10) + BAND * dev
    x = pf & (W - 1)
    tflat = (y >> 2) * (W // KERNEL) + (x >> 2)

    csel = np.where(vf >= T0)[0]
    ok = csel.size >= TOP_K
    if ok:
        order = np.lexsort((tflat[csel], -vf[csel].astype(np.float64)))[:TOP_K]
        sel = csel[order]

        # device ranking replica: key = (floor(v*2^23)>>9)*1024 + pos10
        m = np.floor(vals.astype(np.float64) * (1 << 23)).astype(np.int64)
        grid_p = np.tile(np.repeat(np.arange(128), 64), N_DEV).reshape(N_DEV, 8192)
        yl = poffs >> 10
        xl = poffs & (W - 1)
        col = 8 * ((xl >> 2) >> 5) + ((yl >> 2) & 7)
        kk = (yl & 3) * 4 + (xl & 3)
        key = (m >> 9) * 1024 + col * 16 + kk
        # rank of each slot within its (device, partition): descending by key
        ranks = np.empty((N_DEV, 8192), np.int32)
        kg = key.reshape(N_DEV, 128, 64)
        rk = np.argsort(np.argsort(-kg, axis=2, kind="stable"), axis=2)
        ranks = rk.reshape(N_DEV, 8192)

        sel_dev = sel // 8192
        sel_slot = sel % 8192
        sel_part = sel_slot // 64
        sel_rank = ranks[sel_dev, sel_slot]
        if sel_rank.max() >= NGATH:
            ok = False
        else:
            keypoints = kxy.reshape(-1, 2)[sel].astype(np.float32)
            kptscores = vf[sel].astype(np.float32)
            dstack = np.stack(descs, axis=0)  # [N_DEV, 128*NGATH, C]
            descriptors = dstack[
                sel_dev, sel_part * NGATH + sel_rank
            ].astype(np.float32)

    if not ok:
        return _numpy_reference(scores_map, descriptor_map)
    return keypoints, descriptors, kptscores
